# revision 6
# baseline (speedup 1.0000x reference)
"""Multi-head causal attention (B=4, T=2048, D=1024, H=16) on 8 Trainium2
NeuronCores.

Sharding: core c -> (batch = c//2, head-group = c%2, 8 heads each).
Each core: QKV projection for its batch/head-group, causal attention,
partial output projection over its heads' rows of w_proj, then per-
128-outcol-group 2-way ReduceScatters with its pair core (same batch,
other head-group). Host reassembles with a transpose+concat only.

On-chip orientation is "transposed" throughout (channels on partitions,
tokens on the free dim):
  xT   (D, T)    bf16, pre-transposed on HOST (keeps PE warm from t=0;
                 device transposes ran cold and wasted ~35us)
  qkT  (1024, T) = wqk.T @ xT  (q rows pre-scaled by 1/sqrt(Dh) on host)
  v    (T, 512)  natural, with an extra all-ones column per head slot
  sT   (k, q)    = K_tile @ qT  -> exp on ScalarE -> es (bf16)
  outT (65, q)   = [v|1].T @ es  (fp32 PSUM; row 64 = softmax denom)
  finalT (1024, q) = wp.T @ (outT / denom) + bias -> per-oc ReduceScatter

The QKV projection is produced in 512-token chunks and chunk qc+1 is
woven between the attention head-pairs of q-macro qc, so the PE stream
stays dense (exp latency on ScalarE is hidden by projection matmuls)
and the PE clock stays un-throttled. AV matmuls run one pipeline step
behind scores/exp. Causal mask: strictly-upper triangular 128x128
multiply on boundary tiles only; future k-tiles are never computed.
The output projection emits one 128-outcol partial at a time, each
immediately ReduceScattered (128KB) so the collective stream overlaps
compute instead of serializing 2x512KB ops at the very end.
Matmul operands are bf16 (1 PE cycle/row, fp32 accumulate); softmax
statistics stay fp32; partials/collectives are bf16.
"""

import numpy as np
import ml_dtypes

import concourse.bass as bass
from concourse import bacc
import concourse.mybir as mybir
import concourse.tile as tile
from concourse.bass_utils import run_bass_kernel_spmd
from concourse.masks import make_upper_triangular

B, T, D = 4, 2048, 1024
H_TOT, DH = 16, 64
HL = 8          # heads per core
P = 128
ND = D // P     # 8 d-tiles
NT = T // P     # 16 token tiles
NQ = T // 512   # 4 q-macros
F32 = mybir.dt.float32
BF16 = mybir.dt.bfloat16
AF = mybir.ActivationFunctionType
NP_BF16 = ml_dtypes.bfloat16

REPLICA_GROUPS = [[0, 1], [2, 3], [4, 5], [6, 7]]


def build_bass():
    nc = bacc.Bacc(None, target_bir_lowering=False, num_devices=8)

    xT = nc.dram_tensor("xT", [D, T], BF16, kind="ExternalInput")
    wqk = nc.dram_tensor("wqk", [D, 1024], BF16, kind="ExternalInput")
    wv = nc.dram_tensor("wv", [D, 512], BF16, kind="ExternalInput")
    wp = nc.dram_tensor("wp", [512, D], BF16, kind="ExternalInput")
    bias = nc.dram_tensor("bias", [D], F32, kind="ExternalInput")
    out = nc.dram_tensor("out", [512, T], BF16, kind="ExternalOutput")

    with tile.TileContext(nc, num_cores=8) as tc:
        with (
            tc.tile_pool(name="const", bufs=1) as const_pool,
            tc.tile_pool(name="dram", bufs=1, space="DRAM") as dram_pool,
            tc.tile_pool(name="persist", bufs=1) as persist,
            tc.tile_pool(name="wp_pool", bufs=1) as wp_pool,
            tc.tile_pool(name="es_pool", bufs=10) as es_pool,
            tc.tile_pool(name="oh_pool", bufs=2) as oh_pool,
            tc.tile_pool(name="ohu_pool", bufs=3) as ohu_pool,
            tc.tile_pool(name="cs_pool", bufs=2) as cs_pool,
            tc.tile_pool(name="rb_pool", bufs=3) as rb_pool,
            tc.tile_pool(name="po_pool", bufs=3) as po_pool,
            tc.tile_pool(name="ps_s", bufs=2, space="PSUM") as ps_s,
            tc.tile_pool(name="ps_av", bufs=2, space="PSUM") as ps_av,
            tc.tile_pool(name="ps_mm", bufs=2, space="PSUM") as ps_mm,
        ):
            tri = const_pool.tile([P, P], BF16)
            make_upper_triangular(nc, tri, val=1.0, diag=True)
            bias_sb = const_pool.tile([P, ND], F32)

            qkT = persist.tile([P, ND, T], BF16, name="qkT")
            v1 = persist.tile([P, NT, HL, DH + 1], BF16, name="v1")
            ones_sb = const_pool.tile([P, NT, HL, 1], F32)
            wp_sb = wp_pool.tile([P, 4, D], BF16)

            def attention(qm, weave):
                """Attention for q-macro qm; pulls from `weave` (an iterator
                of thunks emitting projection matmul groups) between pairs."""
                nkt = 4 * qm + 4
                nb = nkt - 4
                oh_sb = oh_pool.tile([P, 4, 512], BF16, name="oh_sb")

                for hp in range(4):
                    heads = (2 * hp, 2 * hp + 1)
                    out_ps = {}
                    for h in heads:
                        out_ps[h] = ps_av.tile([DH + 1, 512], F32,
                                               name=f"out_ps{h % 2}",
                                               tag="ps_av")

                    def av_mm(h, kt, src):
                        nc.tensor.matmul(
                            out_ps[h][:, max(0, P * kt - 512 * qm):],
                            lhsT=v1[:, kt, h, :],
                            rhs=src,
                            start=(kt == 0), stop=(kt == nkt - 1),
                            skip_group_check=True)

                    def scores_mm(h, kt, dst, qoff):
                        hi = (h % 2) * DH
                        nc.tensor.matmul(
                            dst,
                            lhsT=qkT[hi:hi + DH, 4 + h // 2,
                                     kt * P:(kt + 1) * P],
                            rhs=qkT[hi:hi + DH, h // 2,
                                    qm * 512 + qoff:(qm + 1) * 512],
                            start=True, stop=True)

                    # software pipeline: AVs one step behind scores/exp
                    pend = []

                    def flush():
                        for h_, kt_, src_ in pend:
                            av_mm(h_, kt_, src_)
                        pend.clear()

                    for kt2 in range(nb // 2):   # non-boundary, 2 per exp
                        kts = (2 * kt2, 2 * kt2 + 1)
                        step = []
                        for h in heads:
                            s2 = ps_s.tile([P, 2, 512], F32, name="s2",
                                           tag="ps_s")
                            for j, kt in enumerate(kts):
                                scores_mm(h, kt, s2[:, j, :], 0)
                            e2 = es_pool.tile([P, 2, 512], BF16, name="es",
                                              tag="es")
                            nc.scalar.activation(out=e2, in_=s2, func=AF.Exp)
                            for j, kt in enumerate(kts):
                                step.append((h, kt, e2[:, j, :]))
                        flush()
                        pend.extend(step)
                    for kt in range(nb, nkt):    # boundary, masked
                        qoff = P * kt - 512 * qm
                        step = []
                        for h in heads:
                            sb_ps = ps_s.tile([P, 2, 512], F32, name="sb",
                                              tag="ps_s")
                            scores_mm(h, kt, sb_ps[:, 0, qoff:], qoff)
                            e2 = es_pool.tile([P, 2, 512], BF16, name="esb",
                                              tag="es")
                            nc.scalar.activation(
                                out=e2[:, 0, qoff:], in_=sb_ps[:, 0, qoff:],
                                func=AF.Exp)
                            nc.vector.tensor_mul(
                                e2[:, 0, qoff:qoff + P],
                                e2[:, 0, qoff:qoff + P], tri)
                            step.append((h, kt, e2[:, 0, qoff:]))
                        flush()
                        pend.extend(step)
                    flush()

                    # evacuate PSUM accumulators, then normalize
                    ohu = ohu_pool.tile([P, 512], F32, name="ohu")
                    cs = cs_pool.tile([P, 2, 512], F32, name="cs")
                    for i, h in enumerate(heads):
                        hi = (h % 2) * DH
                        nc.vector.tensor_copy(
                            out=ohu[hi:hi + DH, :], in_=out_ps[h][0:DH, :])
                        nc.vector.tensor_copy(
                            out=cs[0:1, i, :], in_=out_ps[h][DH:DH + 1, :])
                    nc.vector.reciprocal(out=cs[0:1, :, :], in_=cs[0:1, :, :])
                    rb = rb_pool.tile([P, 2, 512], F32, name="rb")
                    nc.gpsimd.partition_broadcast(rb[:, 0, :], cs[0:1, 0, :])
                    nc.gpsimd.partition_broadcast(rb[:, 1, :], cs[0:1, 1, :])
                    for i, h in enumerate(heads):
                        hi = (h % 2) * DH
                        nc.vector.tensor_mul(
                            oh_sb[hi:hi + DH, hp, :],
                            ohu[hi:hi + DH, :], rb[hi:hi + DH, i, :])

                    # weave in dense projection work for the next chunk
                    if weave is not None:
                        for _ in range(3):
                            thunk = next(weave, None)
                            if thunk is None:
                                break
                            thunk()
                return oh_sb

            def oc_group(qm, oh_sb, oc):
                """128 output channels: project, bias, 2-way ReduceScatter
                (128KB), write this core's 64-row share to out."""
                ps = ps_mm.tile([P, 512], F32, name="ps_f", tag="ps_mm")
                for dt in range(4):
                    nc.tensor.matmul(
                        ps,
                        lhsT=wp_sb[:, dt, oc * P:(oc + 1) * P],
                        rhs=oh_sb[:, dt, :],
                        start=(dt == 0), stop=(dt == 3))
                po = po_pool.tile([P, 512], BF16, name="po")
                nc.vector.tensor_scalar_add(
                    out=po, in0=ps, scalar1=bias_sb[:, oc:oc + 1])
                partial = dram_pool.tile([P, 512], BF16,
                                         name=f"pa{qm}_{oc}",
                                         tag=f"pa{qm}_{oc}")
                nc.sync.dma_start(out=partial, in_=po)
                rs = dram_pool.tile([64, 512], BF16, name=f"rs{qm}_{oc}",
                                    tag=f"rs{qm}_{oc}")
                nc.gpsimd.collective_compute(
                    "ReduceScatter", mybir.AluOpType.add,
                    replica_groups=REPLICA_GROUPS,
                    ins=[partial[:, :]], outs=[rs[:, :]])
                nc.sync.dma_start(
                    out=out.ap()[64 * oc:64 * (oc + 1),
                                 qm * 512:(qm + 1) * 512], in_=rs)

            def out_proj_groups(qm, oh_sb):
                for oc in range(8):
                    yield (lambda qm=qm, oh=oh_sb, oc=oc:
                           oc_group(qm, oh, oc))

            def out_proj(qm, oh_sb):
                for thunk in out_proj_groups(qm, oh_sb):
                    thunk()

            # ---- projection machinery (chunked by 512 tokens) ----
            with (
                tc.tile_pool(name="xT_pool", bufs=1) as xT_pool,
                tc.tile_pool(name="wa_pool", bufs=1) as wa_pool,
            ):
                # input loads, ordered so the V projection (wv + token
                # chunk 0) can start after ~2MB of DMA
                xT_sb = xT_pool.tile([P, ND, T], BF16)
                xT_src = xT.ap().rearrange("(n p) t -> p n t", p=P)
                for qc in range(4):
                    nc.sync.dma_start(
                        out=xT_sb[:, :, qc * 512:(qc + 1) * 512],
                        in_=xT_src[:, :, qc * 512:(qc + 1) * 512])
                wv_sb = wa_pool.tile([P, ND, 512], BF16)
                nc.gpsimd.dma_start(
                    out=wv_sb, in_=wv.ap().rearrange("(n p) m -> p n m", p=P))
                wqk_sb = wa_pool.tile([P, ND, 1024], BF16)
                nc.gpsimd.dma_start(
                    out=wqk_sb,
                    in_=wqk.ap().rearrange("(n p) m -> p n m", p=P))
                nc.gpsimd.dma_start(
                    out=wp_sb, in_=wp.ap().rearrange("(n p) m -> p n m", p=P))
                nc.gpsimd.dma_start(
                    out=bias_sb, in_=bias.ap().rearrange("(n p) -> p n", p=P))
                nc.vector.memset(ones_sb, 1.0)
                nc.vector.tensor_copy(out=v1[:, :, :, DH:DH + 1], in_=ones_sb)

                def qk_group(pt, qc):
                    ps = ps_mm.tile([P, 512], F32, name="ps_qk", tag="ps_mm")
                    for dd in range(ND):
                        nc.tensor.matmul(
                            ps,
                            lhsT=wqk_sb[:, dd, pt * P:(pt + 1) * P],
                            rhs=xT_sb[:, dd, qc * 512:(qc + 1) * 512],
                            start=(dd == 0), stop=(dd == ND - 1))
                    nc.vector.tensor_copy(
                        out=qkT[:, pt, qc * 512:(qc + 1) * 512], in_=ps)

                def v_group(tt):
                    ps = ps_mm.tile([P, 512], F32, name="ps_v", tag="ps_mm")
                    for dd in range(ND):
                        nc.tensor.matmul(
                            ps,
                            lhsT=xT_sb[:, dd, tt * P:(tt + 1) * P],
                            rhs=wv_sb[:, dd, :],
                            start=(dd == 0), stop=(dd == ND - 1))
                    nc.vector.tensor_copy(
                        out=v1[:, tt, :, 0:DH],
                        in_=ps.rearrange("p (h d) -> p h d", h=HL))

                def proj_chunk_groups(qc):
                    for tt in range(4 * qc, 4 * qc + 4):
                        yield lambda tt=tt: v_group(tt)
                    for pt in range(8):
                        yield lambda pt=pt: qk_group(pt, qc)

                # prelude: project chunk 0 (V first: needs only wv+chunk 0)
                for g in proj_chunk_groups(0):
                    g()

                # q-macros 0..2, weaving in the next chunk's projections
                oh2 = None
                for qm in range(3):
                    weave = proj_chunk_groups(qm + 1)
                    oh_sb = attention(qm, weave)
                    for thunk in weave:  # drain leftovers
                        thunk()
                    if qm < 2:
                        out_proj(qm, oh_sb)
                    else:
                        oh2 = oh_sb

            # q-macro 3: weave qm2's out-projection between its pairs
            weave3 = out_proj_groups(2, oh2)
            oh_sb = attention(3, weave3)
            for thunk in weave3:
                thunk()
            out_proj(3, oh_sb)

    nc.finalize()
    return nc


_NC_CACHE = None


def _get_nc():
    global _NC_CACHE
    if _NC_CACHE is None:
        _NC_CACHE = build_bass()
    return _NC_CACHE


def _make_in_maps(x, w_qkv, w_proj, b_proj):
    x = np.asarray(x, np.float32)
    w_qkv = np.asarray(w_qkv, np.float32)
    w_proj = np.asarray(w_proj, np.float32)
    b_proj = np.asarray(b_proj, np.float32)
    wq, wk, wv_full = w_qkv[:, :D], w_qkv[:, D:2 * D], w_qkv[:, 2 * D:]
    scale = DH ** -0.5
    in_maps = []
    for c in range(8):
        b, g = c // 2, c % 2
        cols = slice(g * 512, (g + 1) * 512)
        wqk_c = np.concatenate([wq[:, cols] * scale, wk[:, cols]], axis=1)
        in_maps.append({
            "xT": np.ascontiguousarray(x[b].T).astype(NP_BF16),
            "wqk": np.ascontiguousarray(wqk_c).astype(NP_BF16),
            "wv": np.ascontiguousarray(wv_full[:, cols]).astype(NP_BF16),
            "wp": np.ascontiguousarray(
                w_proj[g * 512:(g + 1) * 512, :]).astype(NP_BF16),
            "bias": b_proj if g == 0 else np.zeros_like(b_proj),
        })
    return in_maps


def _assemble(results):
    out = np.empty((B, T, D), np.float32)
    for c in range(8):
        b, r = c // 2, c % 2
        res = results[c]["out"].astype(np.float32)
        # out row 64*oc + k  ->  global outcol 128*oc + 64*r + k
        for oc in range(8):
            out[b, :, 128 * oc + 64 * r:128 * oc + 64 * r + 64] = \
                res[64 * oc:64 * (oc + 1)].T
    return out


def kernel(x, w_qkv, w_proj, b_proj):
    nc = _get_nc()
    in_maps = _make_in_maps(x, w_qkv, w_proj, b_proj)
    res = run_bass_kernel_spmd(nc, in_maps, core_ids=list(range(8)))
    return _assemble(res.results)


def kernel_traced(x, w_qkv, w_proj, b_proj, **kw):
    """Like kernel() but returns (output, BassKernelResults) with trace."""
    nc = _get_nc()
    in_maps = _make_in_maps(x, w_qkv, w_proj, b_proj)
    res = run_bass_kernel_spmd(nc, in_maps, core_ids=list(range(8)),
                               trace=True, **kw)
    return _assemble(res.results), res


# revision 7
# speedup vs baseline: 1.0049x; 1.0049x over previous
"""Multi-head causal attention (B=4, T=2048, D=1024, H=16) on 8 Trainium2
NeuronCores.

Sharding: core c -> (batch = c//2, head-group = c%2, 8 heads each).
Each core: QKV projection for its batch/head-group, causal attention,
partial output projection over its heads' rows of w_proj, then per-
128-outcol-group 2-way ReduceScatters with its pair core (same batch,
other head-group). Host reassembles with a transpose+concat only.

On-chip orientation is "transposed" throughout (channels on partitions,
tokens on the free dim):
  xT   (D, T)    bf16, pre-transposed on HOST (keeps PE warm from t=0;
                 device transposes ran cold and wasted ~35us)
  qkT  (1024, T) = wqk.T @ xT  (q rows pre-scaled by 1/sqrt(Dh) on host)
  v    (T, 512)  natural, with an extra all-ones column per head slot
  sT   (k, q)    = K_tile @ qT  -> exp on ScalarE -> es (bf16)
  outT (65, q)   = [v|1].T @ es  (fp32 PSUM; row 64 = softmax denom)
  finalT (1024, q) = wp.T @ (outT / denom) + bias -> per-oc ReduceScatter

The QKV projection is produced in 512-token chunks and chunk qc+1 is
woven between the attention head-pairs of q-macro qc, so the PE stream
stays dense (exp latency on ScalarE is hidden by projection matmuls)
and the PE clock stays un-throttled. AV matmuls run one pipeline step
behind scores/exp. Causal mask: strictly-upper triangular 128x128
multiply on boundary tiles only; future k-tiles are never computed.
The output projection emits one 128-outcol partial at a time, each
immediately ReduceScattered (128KB) so the collective stream overlaps
compute instead of serializing 2x512KB ops at the very end.
Matmul operands are bf16 (1 PE cycle/row, fp32 accumulate); softmax
statistics stay fp32; partials/collectives are bf16.
"""

import numpy as np
import ml_dtypes

import concourse.bass as bass
from concourse import bacc
import concourse.mybir as mybir
import concourse.tile as tile
from concourse.bass_utils import run_bass_kernel_spmd
from concourse.masks import make_upper_triangular

B, T, D = 4, 2048, 1024
H_TOT, DH = 16, 64
HL = 8          # heads per core
P = 128
ND = D // P     # 8 d-tiles
NT = T // P     # 16 token tiles
NQ = T // 512   # 4 q-macros
F32 = mybir.dt.float32
BF16 = mybir.dt.bfloat16
AF = mybir.ActivationFunctionType
NP_BF16 = ml_dtypes.bfloat16

REPLICA_GROUPS = [[0, 1], [2, 3], [4, 5], [6, 7]]


def build_bass():
    nc = bacc.Bacc(None, target_bir_lowering=False, num_devices=8)

    xT = nc.dram_tensor("xT", [D, T], BF16, kind="ExternalInput")
    wqk = nc.dram_tensor("wqk", [D, 1024], BF16, kind="ExternalInput")
    wv = nc.dram_tensor("wv", [D, 512], BF16, kind="ExternalInput")
    wp = nc.dram_tensor("wp", [512, D], BF16, kind="ExternalInput")
    bias = nc.dram_tensor("bias", [D], F32, kind="ExternalInput")
    out = nc.dram_tensor("out", [512, T], BF16, kind="ExternalOutput")

    with tile.TileContext(nc, num_cores=8) as tc:
        with (
            tc.tile_pool(name="const", bufs=1) as const_pool,
            tc.tile_pool(name="dram", bufs=1, space="DRAM") as dram_pool,
            tc.tile_pool(name="persist", bufs=1) as persist,
            tc.tile_pool(name="wp_pool", bufs=1) as wp_pool,
            tc.tile_pool(name="es_pool", bufs=10) as es_pool,
            tc.tile_pool(name="oh_pool", bufs=2) as oh_pool,
            tc.tile_pool(name="ohu_pool", bufs=3) as ohu_pool,
            tc.tile_pool(name="cs_pool", bufs=2) as cs_pool,
            tc.tile_pool(name="rb_pool", bufs=3) as rb_pool,
            tc.tile_pool(name="po_pool", bufs=3) as po_pool,
            tc.tile_pool(name="ps_s", bufs=2, space="PSUM") as ps_s,
            tc.tile_pool(name="ps_av", bufs=2, space="PSUM") as ps_av,
            tc.tile_pool(name="ps_mm", bufs=2, space="PSUM") as ps_mm,
        ):
            tri = const_pool.tile([P, P], BF16)
            make_upper_triangular(nc, tri, val=1.0, diag=True)
            bias_sb = const_pool.tile([P, ND], F32)

            qkT = persist.tile([P, ND, T], BF16, name="qkT")
            v1 = persist.tile([P, NT, HL, DH + 1], BF16, name="v1")
            ones_sb = const_pool.tile([P, NT, HL, 1], F32)
            wp_sb = wp_pool.tile([P, 4, D], BF16)

            def attention(qm, weave):
                """Attention for q-macro qm; pulls from `weave` (an iterator
                of thunks emitting projection matmul groups) between pairs."""
                nkt = 4 * qm + 4
                nb = nkt - 4
                oh_sb = oh_pool.tile([P, 4, 512], BF16, name="oh_sb")

                for hp in range(4):
                    heads = (2 * hp, 2 * hp + 1)
                    out_ps = {}
                    for h in heads:
                        out_ps[h] = ps_av.tile([DH + 1, 512], F32,
                                               name=f"out_ps{h % 2}",
                                               tag="ps_av")

                    def av_mm(h, kt, src):
                        nc.tensor.matmul(
                            out_ps[h][:, max(0, P * kt - 512 * qm):],
                            lhsT=v1[:, kt, h, :],
                            rhs=src,
                            start=(kt == 0), stop=(kt == nkt - 1),
                            skip_group_check=True)

                    def scores_mm(h, kt, dst, qoff):
                        hi = (h % 2) * DH
                        nc.tensor.matmul(
                            dst,
                            lhsT=qkT[hi:hi + DH, 4 + h // 2,
                                     kt * P:(kt + 1) * P],
                            rhs=qkT[hi:hi + DH, h // 2,
                                    qm * 512 + qoff:(qm + 1) * 512],
                            start=True, stop=True)

                    # software pipeline: AVs one step behind scores/exp
                    pend = []

                    def flush():
                        for h_, kt_, src_ in pend:
                            av_mm(h_, kt_, src_)
                        pend.clear()

                    for kt2 in range(nb // 2):   # non-boundary, 2 per exp
                        kts = (2 * kt2, 2 * kt2 + 1)
                        step = []
                        for h in heads:
                            s2 = ps_s.tile([P, 2, 512], F32, name="s2",
                                           tag="ps_s")
                            for j, kt in enumerate(kts):
                                scores_mm(h, kt, s2[:, j, :], 0)
                            e2 = es_pool.tile([P, 2, 512], BF16, name="es",
                                              tag="es")
                            nc.scalar.activation(out=e2, in_=s2, func=AF.Exp)
                            for j, kt in enumerate(kts):
                                step.append((h, kt, e2[:, j, :]))
                        flush()
                        pend.extend(step)
                    for kt in range(nb, nkt):    # boundary, masked
                        qoff = P * kt - 512 * qm
                        step = []
                        for h in heads:
                            sb_ps = ps_s.tile([P, 2, 512], F32, name="sb",
                                              tag="ps_s")
                            scores_mm(h, kt, sb_ps[:, 0, qoff:], qoff)
                            e2 = es_pool.tile([P, 2, 512], BF16, name="esb",
                                              tag="es")
                            nc.scalar.activation(
                                out=e2[:, 0, qoff:], in_=sb_ps[:, 0, qoff:],
                                func=AF.Exp)
                            nc.vector.tensor_mul(
                                e2[:, 0, qoff:qoff + P],
                                e2[:, 0, qoff:qoff + P], tri)
                            step.append((h, kt, e2[:, 0, qoff:]))
                        flush()
                        pend.extend(step)
                    flush()

                    # evacuate PSUM accumulators, then normalize
                    ohu = ohu_pool.tile([P, 512], F32, name="ohu")
                    cs = cs_pool.tile([P, 2, 512], F32, name="cs")
                    for i, h in enumerate(heads):
                        hi = (h % 2) * DH
                        nc.vector.tensor_copy(
                            out=ohu[hi:hi + DH, :], in_=out_ps[h][0:DH, :])
                        nc.vector.tensor_copy(
                            out=cs[0:1, i, :], in_=out_ps[h][DH:DH + 1, :])
                    csw = cs_pool.tile([P, 8], F32, name="csw", tag="csw")
                    nc.sync.dma_start(out=csw, in_=cs[0:1, :, :])
                    nc.vector.reciprocal(out=csw, in_=csw)
                    rcs = cs_pool.tile([P, 2, 512], F32, name="rcs", tag="rcs")
                    nc.sync.dma_start(out=rcs[0:1, :, :], in_=csw)
                    rb = rb_pool.tile([P, 2, 512], F32, name="rb")
                    nc.gpsimd.partition_broadcast(rb[:, 0, :], rcs[0:1, 0, :])
                    nc.gpsimd.partition_broadcast(rb[:, 1, :], rcs[0:1, 1, :])
                    for i, h in enumerate(heads):
                        hi = (h % 2) * DH
                        nc.vector.tensor_mul(
                            oh_sb[hi:hi + DH, hp, :],
                            ohu[hi:hi + DH, :], rb[hi:hi + DH, i, :])

                    # weave in dense projection work for the next chunk
                    if weave is not None:
                        for _ in range(3):
                            thunk = next(weave, None)
                            if thunk is None:
                                break
                            thunk()
                return oh_sb

            def oc_group(qm, oh_sb, oc):
                """128 output channels: project, bias, 2-way ReduceScatter
                (128KB), write this core's 64-row share to out."""
                ps = ps_mm.tile([P, 512], F32, name="ps_f", tag="ps_mm")
                for dt in range(4):
                    nc.tensor.matmul(
                        ps,
                        lhsT=wp_sb[:, dt, oc * P:(oc + 1) * P],
                        rhs=oh_sb[:, dt, :],
                        start=(dt == 0), stop=(dt == 3))
                po = po_pool.tile([P, 512], BF16, name="po")
                nc.vector.tensor_scalar_add(
                    out=po, in0=ps, scalar1=bias_sb[:, oc:oc + 1])
                partial = dram_pool.tile([P, 512], BF16,
                                         name=f"pa{qm}_{oc}",
                                         tag=f"pa{qm}_{oc}")
                nc.sync.dma_start(out=partial, in_=po)
                rs = dram_pool.tile([64, 512], BF16, name=f"rs{qm}_{oc}",
                                    tag=f"rs{qm}_{oc}")
                nc.gpsimd.collective_compute(
                    "ReduceScatter", mybir.AluOpType.add,
                    replica_groups=REPLICA_GROUPS,
                    ins=[partial[:, :]], outs=[rs[:, :]])
                nc.sync.dma_start(
                    out=out.ap()[64 * oc:64 * (oc + 1),
                                 qm * 512:(qm + 1) * 512], in_=rs)

            def out_proj_groups(qm, oh_sb):
                for oc in range(8):
                    yield (lambda qm=qm, oh=oh_sb, oc=oc:
                           oc_group(qm, oh, oc))

            def out_proj(qm, oh_sb):
                for thunk in out_proj_groups(qm, oh_sb):
                    thunk()

            # ---- projection machinery (chunked by 512 tokens) ----
            with (
                tc.tile_pool(name="xT_pool", bufs=1) as xT_pool,
                tc.tile_pool(name="wa_pool", bufs=1) as wa_pool,
            ):
                # input loads, ordered so the V projection (wv + token
                # chunk 0) can start after ~2MB of DMA
                xT_sb = xT_pool.tile([P, ND, T], BF16)
                xT_src = xT.ap().rearrange("(n p) t -> p n t", p=P)
                for qc in range(4):
                    nc.sync.dma_start(
                        out=xT_sb[:, :, qc * 512:(qc + 1) * 512],
                        in_=xT_src[:, :, qc * 512:(qc + 1) * 512])
                wv_sb = wa_pool.tile([P, ND, 512], BF16)
                nc.gpsimd.dma_start(
                    out=wv_sb, in_=wv.ap().rearrange("(n p) m -> p n m", p=P))
                wqk_sb = wa_pool.tile([P, ND, 1024], BF16)
                nc.gpsimd.dma_start(
                    out=wqk_sb,
                    in_=wqk.ap().rearrange("(n p) m -> p n m", p=P))
                nc.gpsimd.dma_start(
                    out=wp_sb, in_=wp.ap().rearrange("(n p) m -> p n m", p=P))
                nc.gpsimd.dma_start(
                    out=bias_sb, in_=bias.ap().rearrange("(n p) -> p n", p=P))
                nc.vector.memset(ones_sb, 1.0)
                nc.vector.tensor_copy(out=v1[:, :, :, DH:DH + 1], in_=ones_sb)

                def qk_group(pt, qc):
                    ps = ps_mm.tile([P, 512], F32, name="ps_qk", tag="ps_mm")
                    for dd in range(ND):
                        nc.tensor.matmul(
                            ps,
                            lhsT=wqk_sb[:, dd, pt * P:(pt + 1) * P],
                            rhs=xT_sb[:, dd, qc * 512:(qc + 1) * 512],
                            start=(dd == 0), stop=(dd == ND - 1))
                    nc.vector.tensor_copy(
                        out=qkT[:, pt, qc * 512:(qc + 1) * 512], in_=ps)

                def v_group(tt):
                    ps = ps_mm.tile([P, 512], F32, name="ps_v", tag="ps_mm")
                    for dd in range(ND):
                        nc.tensor.matmul(
                            ps,
                            lhsT=xT_sb[:, dd, tt * P:(tt + 1) * P],
                            rhs=wv_sb[:, dd, :],
                            start=(dd == 0), stop=(dd == ND - 1))
                    nc.vector.tensor_copy(
                        out=v1[:, tt, :, 0:DH],
                        in_=ps.rearrange("p (h d) -> p h d", h=HL))

                def proj_chunk_groups(qc):
                    for tt in range(4 * qc, 4 * qc + 4):
                        yield lambda tt=tt: v_group(tt)
                    for pt in range(8):
                        yield lambda pt=pt: qk_group(pt, qc)

                # prelude: project chunk 0 (V first: needs only wv+chunk 0)
                for g in proj_chunk_groups(0):
                    g()

                # q-macros 0..2, weaving in the next chunk's projections
                oh2 = None
                for qm in range(3):
                    weave = proj_chunk_groups(qm + 1)
                    oh_sb = attention(qm, weave)
                    for thunk in weave:  # drain leftovers
                        thunk()
                    if qm < 2:
                        out_proj(qm, oh_sb)
                    else:
                        oh2 = oh_sb

            # q-macro 3: weave qm2's out-projection between its pairs
            weave3 = out_proj_groups(2, oh2)
            oh_sb = attention(3, weave3)
            for thunk in weave3:
                thunk()
            out_proj(3, oh_sb)

    nc.finalize()
    return nc


_NC_CACHE = None


def _get_nc():
    global _NC_CACHE
    if _NC_CACHE is None:
        _NC_CACHE = build_bass()
    return _NC_CACHE


def _make_in_maps(x, w_qkv, w_proj, b_proj):
    x = np.asarray(x, np.float32)
    w_qkv = np.asarray(w_qkv, np.float32)
    w_proj = np.asarray(w_proj, np.float32)
    b_proj = np.asarray(b_proj, np.float32)
    wq, wk, wv_full = w_qkv[:, :D], w_qkv[:, D:2 * D], w_qkv[:, 2 * D:]
    scale = DH ** -0.5
    in_maps = []
    for c in range(8):
        b, g = c // 2, c % 2
        cols = slice(g * 512, (g + 1) * 512)
        wqk_c = np.concatenate([wq[:, cols] * scale, wk[:, cols]], axis=1)
        in_maps.append({
            "xT": np.ascontiguousarray(x[b].T).astype(NP_BF16),
            "wqk": np.ascontiguousarray(wqk_c).astype(NP_BF16),
            "wv": np.ascontiguousarray(wv_full[:, cols]).astype(NP_BF16),
            "wp": np.ascontiguousarray(
                w_proj[g * 512:(g + 1) * 512, :]).astype(NP_BF16),
            "bias": b_proj if g == 0 else np.zeros_like(b_proj),
        })
    return in_maps


def _assemble(results):
    out = np.empty((B, T, D), np.float32)
    for c in range(8):
        b, r = c // 2, c % 2
        res = results[c]["out"].astype(np.float32)
        # out row 64*oc + k  ->  global outcol 128*oc + 64*r + k
        for oc in range(8):
            out[b, :, 128 * oc + 64 * r:128 * oc + 64 * r + 64] = \
                res[64 * oc:64 * (oc + 1)].T
    return out


def kernel(x, w_qkv, w_proj, b_proj):
    nc = _get_nc()
    in_maps = _make_in_maps(x, w_qkv, w_proj, b_proj)
    res = run_bass_kernel_spmd(nc, in_maps, core_ids=list(range(8)))
    return _assemble(res.results)


def kernel_traced(x, w_qkv, w_proj, b_proj, **kw):
    """Like kernel() but returns (output, BassKernelResults) with trace."""
    nc = _get_nc()
    in_maps = _make_in_maps(x, w_qkv, w_proj, b_proj)
    res = run_bass_kernel_spmd(nc, in_maps, core_ids=list(range(8)),
                               trace=True, **kw)
    return _assemble(res.results), res


# revision 9
# speedup vs baseline: 1.1721x; 1.1664x over previous
"""Multi-head causal attention (B=4, T=2048, D=1024, H=16) on 8 Trainium2
NeuronCores.

Sharding: core c -> (batch = c//2, head-group g = c%2, 8 heads each).
Each core: QKV projection for its batch/head-group, causal attention,
then a per-head-pair 2-way AllGather of the attention output with its
pair core (same batch, other head-group), followed by a fully LOCAL
output projection over this core's 512 output channels (full 1024-
channel contraction) written straight to out. No ReduceScatter, no
partial staging: the only collective after the last attention matmul
is one 128KB AllGather. Host reassembles with a transpose+concat only.

On-chip orientation is "transposed" throughout (channels on partitions,
tokens on the free dim):
  xT   (D, T)    bf16, pre-transposed on HOST (keeps PE warm from t=0)
  qkT  (1024, T) = wqk.T @ xT  (q rows pre-scaled by 1/sqrt(Dh) on host)
  v    (T, 512)  natural, with an extra all-ones column per head slot
  sT   (k, q)    = K_tile @ qT  -> exp on ScalarE -> es (bf16)
  outT (65, q)   = [v|1].T @ es  (fp32 PSUM; row 64 = softmax denom)
  oh   (512, q)  normalized attention out, AllGathered per 128-row
                 head-pair slab as soon as each head-pair completes
  final (512own, q) = wp_all.T @ ohAG + bias  -> DMA to out

The QKV projection is produced in 512-token chunks and chunk qc+1 is
woven between the attention head-pairs of q-macro qc, so the PE stream
stays dense (exp latency on ScalarE is hidden by projection matmuls)
and the PE clock stays un-throttled. AV matmuls run one pipeline step
behind scores/exp. Causal mask: strictly-upper triangular 128x128
multiply on boundary tiles only; future k-tiles are never computed.
Matmul operands are bf16 (1 PE cycle/row, fp32 accumulate); softmax
statistics stay fp32; exchanged slabs are bf16.
"""

import numpy as np
import ml_dtypes

import concourse.bass as bass
from concourse import bacc
import concourse.mybir as mybir
import concourse.tile as tile
from concourse.bass_utils import run_bass_kernel_spmd
from concourse.masks import make_upper_triangular

B, T, D = 4, 2048, 1024
H_TOT, DH = 16, 64
HL = 8          # heads per core
P = 128
ND = D // P     # 8 d-tiles
NT = T // P     # 16 token tiles
NQ = T // 512   # 4 q-macros
F32 = mybir.dt.float32
BF16 = mybir.dt.bfloat16
AF = mybir.ActivationFunctionType
NP_BF16 = ml_dtypes.bfloat16

REPLICA_GROUPS = [[0, 1], [2, 3], [4, 5], [6, 7]]


def build_bass():
    nc = bacc.Bacc(None, target_bir_lowering=False, num_devices=8)

    xT = nc.dram_tensor("xT", [D, T], BF16, kind="ExternalInput")
    wqk = nc.dram_tensor("wqk", [D, 1024], BF16, kind="ExternalInput")
    wv = nc.dram_tensor("wv", [D, 512], BF16, kind="ExternalInput")
    # full 1024 contraction rows x this core's 512 output cols
    wp = nc.dram_tensor("wp", [D, 512], BF16, kind="ExternalInput")
    bias = nc.dram_tensor("bias", [512], F32, kind="ExternalInput")
    out = nc.dram_tensor("out", [512, T], BF16, kind="ExternalOutput")

    with tile.TileContext(nc, num_cores=8) as tc:
        with (
            tc.tile_pool(name="const", bufs=1) as const_pool,
            tc.tile_pool(name="dram", bufs=1, space="DRAM") as dram_pool,
            tc.tile_pool(name="persist", bufs=1) as persist,
            tc.tile_pool(name="wp_pool", bufs=1) as wp_pool,
            tc.tile_pool(name="es_pool", bufs=10) as es_pool,
            tc.tile_pool(name="oh_pool", bufs=2) as oh_pool,
            tc.tile_pool(name="og_pool", bufs=2) as og_pool,
            tc.tile_pool(name="ohu_pool", bufs=3) as ohu_pool,
            tc.tile_pool(name="cs_pool", bufs=2) as cs_pool,
            tc.tile_pool(name="rb_pool", bufs=3) as rb_pool,
            tc.tile_pool(name="po_pool", bufs=3) as po_pool,
            tc.tile_pool(name="ps_s", bufs=2, space="PSUM") as ps_s,
            tc.tile_pool(name="ps_av", bufs=2, space="PSUM") as ps_av,
            tc.tile_pool(name="ps_mm", bufs=2, space="PSUM") as ps_mm,
        ):
            tri = const_pool.tile([P, P], BF16)
            make_upper_triangular(nc, tri, val=1.0, diag=True)
            bias_sb = const_pool.tile([P, 4], F32)

            qkT = persist.tile([P, ND, T], BF16, name="qkT")
            v1 = persist.tile([P, NT, HL, DH + 1], BF16, name="v1")
            ones_sb = const_pool.tile([P, NT, HL, 1], F32)
            # wp2[p, rank, hp, c] = w_proj[rank*512 + hp*128 + p, mycols[c]]
            wp2_sb = wp_pool.tile([P, 2, 4, 512], BF16)

            def attention(qm, weave):
                """Attention for q-macro qm; pulls from `weave` (an iterator
                of thunks emitting projection matmul groups) between pairs.
                Each finished head-pair slab is AllGathered with the pair
                core into ohg_sb[:, rank, hp, :]."""
                nkt = 4 * qm + 4
                nb = nkt - 4
                oh_sb = oh_pool.tile([P, 4, 512], BF16, name="oh_sb")
                ohg_sb = og_pool.tile([P, 2, 4, 512], BF16, name="ohg_sb")

                for hp in range(4):
                    heads = (2 * hp, 2 * hp + 1)
                    out_ps = {}
                    for h in heads:
                        out_ps[h] = ps_av.tile([DH + 1, 512], F32,
                                               name=f"out_ps{h % 2}",
                                               tag="ps_av")

                    def av_mm(h, kt, src):
                        nc.tensor.matmul(
                            out_ps[h][:, max(0, P * kt - 512 * qm):],
                            lhsT=v1[:, kt, h, :],
                            rhs=src,
                            start=(kt == 0), stop=(kt == nkt - 1),
                            skip_group_check=True)

                    def scores_mm(h, kt, dst, qoff):
                        hi = (h % 2) * DH
                        nc.tensor.matmul(
                            dst,
                            lhsT=qkT[hi:hi + DH, 4 + h // 2,
                                     kt * P:(kt + 1) * P],
                            rhs=qkT[hi:hi + DH, h // 2,
                                    qm * 512 + qoff:(qm + 1) * 512],
                            start=True, stop=True)

                    # software pipeline: AVs one step behind scores/exp
                    pend = []

                    def flush():
                        for h_, kt_, src_ in pend:
                            av_mm(h_, kt_, src_)
                        pend.clear()

                    for kt2 in range(nb // 2):   # non-boundary, 2 per exp
                        kts = (2 * kt2, 2 * kt2 + 1)
                        step = []
                        for h in heads:
                            s2 = ps_s.tile([P, 2, 512], F32, name="s2",
                                           tag="ps_s")
                            for j, kt in enumerate(kts):
                                scores_mm(h, kt, s2[:, j, :], 0)
                            e2 = es_pool.tile([P, 2, 512], BF16, name="es",
                                              tag="es")
                            nc.scalar.activation(out=e2, in_=s2, func=AF.Exp)
                            for j, kt in enumerate(kts):
                                step.append((h, kt, e2[:, j, :]))
                        flush()
                        pend.extend(step)
                    for kt in range(nb, nkt):    # boundary, masked
                        qoff = P * kt - 512 * qm
                        step = []
                        for h in heads:
                            sb_ps = ps_s.tile([P, 2, 512], F32, name="sb",
                                              tag="ps_s")
                            scores_mm(h, kt, sb_ps[:, 0, qoff:], qoff)
                            e2 = es_pool.tile([P, 2, 512], BF16, name="esb",
                                              tag="es")
                            nc.scalar.activation(
                                out=e2[:, 0, qoff:], in_=sb_ps[:, 0, qoff:],
                                func=AF.Exp)
                            nc.vector.tensor_mul(
                                e2[:, 0, qoff:qoff + P],
                                e2[:, 0, qoff:qoff + P], tri)
                            step.append((h, kt, e2[:, 0, qoff:]))
                        flush()
                        pend.extend(step)
                    flush()

                    # evacuate PSUM accumulators, then normalize
                    ohu = ohu_pool.tile([P, 512], F32, name="ohu")
                    cs = cs_pool.tile([P, 2, 512], F32, name="cs")
                    for i, h in enumerate(heads):
                        hi = (h % 2) * DH
                        nc.vector.tensor_copy(
                            out=ohu[hi:hi + DH, :], in_=out_ps[h][0:DH, :])
                        nc.vector.tensor_copy(
                            out=cs[0:1, i, :], in_=out_ps[h][DH:DH + 1, :])
                    csw = cs_pool.tile([P, 8], F32, name="csw", tag="csw")
                    nc.sync.dma_start(out=csw, in_=cs[0:1, :, :])
                    nc.vector.reciprocal(out=csw, in_=csw)
                    rcs = cs_pool.tile([P, 2, 512], F32, name="rcs", tag="rcs")
                    nc.sync.dma_start(out=rcs[0:1, :, :], in_=csw)
                    rb = rb_pool.tile([P, 2, 512], F32, name="rb")
                    nc.gpsimd.partition_broadcast(rb[:, 0, :], rcs[0:1, 0, :])
                    nc.gpsimd.partition_broadcast(rb[:, 1, :], rcs[0:1, 1, :])
                    for i, h in enumerate(heads):
                        hi = (h % 2) * DH
                        nc.vector.tensor_mul(
                            oh_sb[hi:hi + DH, hp, :],
                            ohu[hi:hi + DH, :], rb[hi:hi + DH, i, :])

                    # exchange this head-pair slab with the pair core
                    ohd = dram_pool.tile([P, 512], BF16,
                                         name=f"ohd{qm}_{hp}",
                                         tag=f"ohd{qm}_{hp}")
                    nc.sync.dma_start(out=ohd, in_=oh_sb[:, hp, :])
                    ohg = dram_pool.tile([2 * P, 512], BF16,
                                         name=f"ohg{qm}_{hp}",
                                         tag=f"ohg{qm}_{hp}")
                    nc.gpsimd.collective_compute(
                        "AllGather", mybir.AluOpType.bypass,
                        replica_groups=REPLICA_GROUPS,
                        ins=[ohd[:, :]], outs=[ohg[:, :]])
                    nc.sync.dma_start(
                        out=ohg_sb[:, :, hp, :],
                        in_=ohg.rearrange("(r p) q -> p r q", p=P))

                    # weave in dense projection work for the next chunk
                    if weave is not None:
                        for _ in range(4):
                            thunk = next(weave, None)
                            if thunk is None:
                                break
                            thunk()
                return ohg_sb

            def oc_group(qm, ohg_sb, oc):
                """This core's output cols [128*oc, 128*oc+128): full
                1024-channel contraction over both ranks' slabs."""
                ps = ps_mm.tile([P, 512], F32, name="ps_f", tag="ps_mm")
                first = True
                for rank in range(2):
                    for hp in range(4):
                        nc.tensor.matmul(
                            ps,
                            lhsT=wp2_sb[:, rank, hp, oc * P:(oc + 1) * P],
                            rhs=ohg_sb[:, rank, hp, :],
                            start=first, stop=(rank == 1 and hp == 3))
                        first = False
                po = po_pool.tile([P, 512], BF16, name="po")
                nc.vector.tensor_scalar_add(
                    out=po, in0=ps, scalar1=bias_sb[:, oc:oc + 1])
                nc.sync.dma_start(
                    out=out.ap()[P * oc:P * (oc + 1),
                                 qm * 512:(qm + 1) * 512], in_=po)

            def out_proj_groups(qm, ohg_sb):
                for oc in range(4):
                    yield (lambda qm=qm, og=ohg_sb, oc=oc:
                           oc_group(qm, og, oc))

            def out_proj(qm, ohg_sb):
                for thunk in out_proj_groups(qm, ohg_sb):
                    thunk()

            # ---- projection machinery (chunked by 512 tokens) ----
            with (
                tc.tile_pool(name="xT_pool", bufs=1) as xT_pool,
                tc.tile_pool(name="wa_pool", bufs=1) as wa_pool,
            ):
                # input loads, ordered so the V projection (wv + token
                # chunk 0) can start after ~2MB of DMA
                xT_sb = xT_pool.tile([P, ND, T], BF16)
                xT_src = xT.ap().rearrange("(n p) t -> p n t", p=P)
                for qc in range(4):
                    nc.sync.dma_start(
                        out=xT_sb[:, :, qc * 512:(qc + 1) * 512],
                        in_=xT_src[:, :, qc * 512:(qc + 1) * 512])
                wv_sb = wa_pool.tile([P, ND, 512], BF16)
                nc.gpsimd.dma_start(
                    out=wv_sb, in_=wv.ap().rearrange("(n p) m -> p n m", p=P))
                wqk_sb = wa_pool.tile([P, ND, 1024], BF16)
                nc.gpsimd.dma_start(
                    out=wqk_sb,
                    in_=wqk.ap().rearrange("(n p) m -> p n m", p=P))
                nc.gpsimd.dma_start(
                    out=wp2_sb,
                    in_=wp.ap().rearrange("(r d p) c -> p r d c", r=2, p=P))
                nc.gpsimd.dma_start(
                    out=bias_sb, in_=bias.ap().rearrange("(n p) -> p n", p=P))
                nc.vector.memset(ones_sb, 1.0)
                nc.vector.tensor_copy(out=v1[:, :, :, DH:DH + 1], in_=ones_sb)

                def qk_group(pt, qc):
                    ps = ps_mm.tile([P, 512], F32, name="ps_qk", tag="ps_mm")
                    for dd in range(ND):
                        nc.tensor.matmul(
                            ps,
                            lhsT=wqk_sb[:, dd, pt * P:(pt + 1) * P],
                            rhs=xT_sb[:, dd, qc * 512:(qc + 1) * 512],
                            start=(dd == 0), stop=(dd == ND - 1))
                    nc.vector.tensor_copy(
                        out=qkT[:, pt, qc * 512:(qc + 1) * 512], in_=ps)

                def v_group(tt):
                    ps = ps_mm.tile([P, 512], F32, name="ps_v", tag="ps_mm")
                    for dd in range(ND):
                        nc.tensor.matmul(
                            ps,
                            lhsT=xT_sb[:, dd, tt * P:(tt + 1) * P],
                            rhs=wv_sb[:, dd, :],
                            start=(dd == 0), stop=(dd == ND - 1))
                    nc.vector.tensor_copy(
                        out=v1[:, tt, :, 0:DH],
                        in_=ps.rearrange("p (h d) -> p h d", h=HL))

                def proj_chunk_groups(qc):
                    for tt in range(4 * qc, 4 * qc + 4):
                        yield lambda tt=tt: v_group(tt)
                    for pt in range(8):
                        yield lambda pt=pt: qk_group(pt, qc)

                # prelude: project chunk 0 (V first: needs only wv+chunk 0)
                for g in proj_chunk_groups(0):
                    g()

                # q-macros 0..2, weaving in the next chunk's projections
                og2 = None
                for qm in range(3):
                    weave = proj_chunk_groups(qm + 1)
                    ohg_sb = attention(qm, weave)
                    for thunk in weave:  # drain leftovers
                        thunk()
                    if qm < 2:
                        out_proj(qm, ohg_sb)
                    else:
                        og2 = ohg_sb

            # q-macro 3: weave qm2's out-projection between its pairs
            weave3 = out_proj_groups(2, og2)
            ohg_sb = attention(3, weave3)
            for thunk in weave3:
                thunk()
            out_proj(3, ohg_sb)

    nc.finalize()
    return nc


_NC_CACHE = None


def _get_nc():
    global _NC_CACHE
    if _NC_CACHE is None:
        _NC_CACHE = build_bass()
    return _NC_CACHE


def _make_in_maps(x, w_qkv, w_proj, b_proj):
    x = np.asarray(x, np.float32)
    w_qkv = np.asarray(w_qkv, np.float32)
    w_proj = np.asarray(w_proj, np.float32)
    b_proj = np.asarray(b_proj, np.float32)
    wq, wk, wv_full = w_qkv[:, :D], w_qkv[:, D:2 * D], w_qkv[:, 2 * D:]
    scale = DH ** -0.5
    in_maps = []
    for c in range(8):
        b, g = c // 2, c % 2
        cols = slice(g * 512, (g + 1) * 512)
        wqk_c = np.concatenate([wq[:, cols] * scale, wk[:, cols]], axis=1)
        in_maps.append({
            "xT": np.ascontiguousarray(x[b].T).astype(NP_BF16),
            "wqk": np.ascontiguousarray(wqk_c).astype(NP_BF16),
            "wv": np.ascontiguousarray(wv_full[:, cols]).astype(NP_BF16),
            "wp": np.ascontiguousarray(w_proj[:, cols]).astype(NP_BF16),
            "bias": np.ascontiguousarray(b_proj[cols]),
        })
    return in_maps


def _assemble(results):
    out = np.empty((B, T, D), np.float32)
    for c in range(8):
        b, g = c // 2, c % 2
        res = results[c]["out"].astype(np.float32)
        out[b, :, g * 512:(g + 1) * 512] = res.T
    return out


def kernel(x, w_qkv, w_proj, b_proj):
    nc = _get_nc()
    in_maps = _make_in_maps(x, w_qkv, w_proj, b_proj)
    res = run_bass_kernel_spmd(nc, in_maps, core_ids=list(range(8)))
    return _assemble(res.results)


def kernel_traced(x, w_qkv, w_proj, b_proj, **kw):
    """Like kernel() but returns (output, BassKernelResults) with trace."""
    nc = _get_nc()
    in_maps = _make_in_maps(x, w_qkv, w_proj, b_proj)
    res = run_bass_kernel_spmd(nc, in_maps, core_ids=list(range(8)),
                               trace=True, **kw)
    return _assemble(res.results), res


# revision 19
# speedup vs baseline: 1.3954x; 1.1905x over previous
"""Multi-head causal attention (B=4, T=2048, D=1024, H=16) on 8 Trainium2
NeuronCores.

Sharding: core c -> (batch = c//2, head-group g = c%2, 8 heads each).
Each core: QKV projection for its batch/head-group, causal attention,
then a per-head-pair 2-way AllGather of the attention output with its
pair core (same batch, other head-group), followed by a fully LOCAL
output projection over this core's 512 output channels (full 1024-
channel contraction) written straight to out. No ReduceScatter, no
partial staging: the only collective after the last attention matmul
is one 128KB AllGather. Host reassembles with a transpose+concat only.

On-chip orientation is "transposed" throughout (channels on partitions,
tokens on the free dim):
  xT   (D, T)    bf16, pre-transposed on HOST (keeps PE warm from t=0)
  qkT  (1024, T) = wqk.T @ xT  (q rows pre-scaled by 1/sqrt(Dh) on host)
  v    (T, 512)  natural, with an extra all-ones column per head slot
  sT   (k, q)    = K_tile @ qT  -> exp on ScalarE -> es (bf16)
  outT (65, q)   = [v|1].T @ es  (fp32 PSUM; row 64 = softmax denom)
  oh   (512, q)  normalized attention out, AllGathered per 128-row
                 head-pair slab as soon as each head-pair completes
  final (512own, q) = wp_all.T @ ohAG + bias  -> DMA to out

The QKV projection is produced in 512-token chunks and chunk qc+1 is
woven between the attention head-pairs of q-macro qc, so the PE stream
stays dense (exp latency on ScalarE is hidden by projection matmuls)
and the PE clock stays un-throttled. AV matmuls run one pipeline step
behind scores/exp. Causal mask: strictly-upper triangular 128x128
multiply on boundary tiles only; future k-tiles are never computed.
Matmul operands are bf16 (1 PE cycle/row, fp32 accumulate); softmax
statistics stay fp32; exchanged slabs are bf16.
"""

import numpy as np
import ml_dtypes

import concourse.bass as bass
from concourse import bacc
import concourse.mybir as mybir
import concourse.tile as tile
from concourse.bass_utils import run_bass_kernel_spmd
from concourse.masks import make_upper_triangular

B, T, D = 4, 2048, 1024
H_TOT, DH = 16, 64
HL = 8          # heads per core
P = 128
ND = D // P     # 8 d-tiles
NT = T // P     # 16 token tiles
NQ = T // 512   # 4 q-macros
F32 = mybir.dt.float32
BF16 = mybir.dt.bfloat16
AF = mybir.ActivationFunctionType
NP_BF16 = ml_dtypes.bfloat16

REPLICA_GROUPS = [[0, 1], [2, 3], [4, 5], [6, 7]]


def build_bass():
    nc = bacc.Bacc(None, target_bir_lowering=False, num_devices=8)

    # all inputs pre-permuted on host into SBUF layout (partition-major)
    # so every input DMA is a contiguous read
    xT = nc.dram_tensor("xT", [4, P, ND, 512], BF16, kind="ExternalInput")
    wqk = nc.dram_tensor("wqk", [P, ND, 1024], BF16, kind="ExternalInput")
    wv = nc.dram_tensor("wv", [P, ND, 512], BF16, kind="ExternalInput")
    # full 1024 contraction rows x this core's 512 output cols
    wp = nc.dram_tensor("wp", [P, 2, 4, 512], BF16, kind="ExternalInput")
    bias = nc.dram_tensor("bias", [P, 4], F32, kind="ExternalInput")
    out = nc.dram_tensor("out", [512, T], BF16, kind="ExternalOutput")

    with tile.TileContext(nc, num_cores=8) as tc:
        with (
            tc.tile_pool(name="const", bufs=1) as const_pool,
            tc.tile_pool(name="dram", bufs=1, space="DRAM") as dram_pool,
            tc.tile_pool(name="persist", bufs=1) as persist,
            tc.tile_pool(name="wp_pool", bufs=1) as wp_pool,
            tc.tile_pool(name="es_pool", bufs=10) as es_pool,
            tc.tile_pool(name="oh_pool", bufs=2) as oh_pool,
            tc.tile_pool(name="og_pool", bufs=2) as og_pool,
            tc.tile_pool(name="ohu_pool", bufs=3) as ohu_pool,
            tc.tile_pool(name="cs_pool", bufs=2) as cs_pool,
            tc.tile_pool(name="rb_pool", bufs=3) as rb_pool,
            tc.tile_pool(name="po_pool", bufs=3) as po_pool,
            tc.tile_pool(name="ps_s", bufs=2, space="PSUM") as ps_s,
            tc.tile_pool(name="ps_av", bufs=2, space="PSUM") as ps_av,
            tc.tile_pool(name="ps_mm", bufs=2, space="PSUM") as ps_mm,
        ):
            tri = const_pool.tile([P, P], BF16)
            make_upper_triangular(nc, tri, val=1.0, diag=True)
            bias_sb = const_pool.tile([P, 4], F32)

            qkT = persist.tile([P, ND, T], BF16, name="qkT")
            v1 = persist.tile([P, NT, HL, DH + 1], BF16, name="v1")
            ones_sb = const_pool.tile([P, NT, HL, 1], F32)
            # wp2[p, rank, hp, c] = w_proj[rank*512 + hp*128 + p, mycols[c]]
            wp2_sb = wp_pool.tile([P, 2, 4, 512], BF16)

            def attention(qm, weave, ag_batch=False):
                """Attention for q-macro qm; pulls from `weave` (an iterator
                of thunks emitting projection matmul groups) between pairs.
                Each finished head-pair slab is AllGathered with the pair
                core into ohg_sb[:, rank, hp, :] (one batched AG at the end
                when ag_batch, to keep the tail off the serialized CC
                stream)."""
                nkt = 4 * qm + 4
                nb = nkt - 4
                oh_sb = oh_pool.tile([P, 4, 512], BF16, name="oh_sb")
                ohg_sb = og_pool.tile([P, 2, 4, 512], BF16, name="ohg_sb")
                ohd4 = None
                if ag_batch:
                    ohd4 = dram_pool.tile([4 * P, 512], BF16,
                                          name=f"ohd4_{qm}", tag=f"ohd4_{qm}")

                for hp in range(4):
                    heads = (2 * hp, 2 * hp + 1)
                    out_ps = {}
                    av_issued = {}
                    for h in heads:
                        out_ps[h] = ps_av.tile([DH + 1, 512], F32,
                                               name=f"out_ps{h % 2}",
                                               tag="ps_av")
                        av_issued[h] = 0

                    def av_mm(h, kt, src):
                        nc.tensor.matmul(
                            out_ps[h][:, max(0, P * kt - 512 * qm):],
                            lhsT=v1[:, kt, h, :],
                            rhs=src,
                            start=(av_issued[h] == 0),
                            stop=(av_issued[h] == nkt - 1),
                            skip_group_check=True)
                        av_issued[h] += 1

                    def scores_mm(h, kt, dst, qoff):
                        hi = (h % 2) * DH
                        nc.tensor.matmul(
                            dst,
                            lhsT=qkT[hi:hi + DH, 4 + h // 2,
                                     kt * P:(kt + 1) * P],
                            rhs=qkT[hi:hi + DH, h // 2,
                                    qm * 512 + qoff:(qm + 1) * 512],
                            start=True, stop=True)

                    # software pipeline: AVs one step behind scores/exp
                    pend = []

                    def flush():
                        for h_, kt_, src_ in pend:
                            av_mm(h_, kt_, src_)
                        pend.clear()

                    for kt2 in range(nb // 2):   # non-boundary, 2 per exp
                        kts = (2 * kt2, 2 * kt2 + 1)
                        step = []
                        for h in heads:
                            s2 = ps_s.tile([P, 2, 512], F32, name="s2",
                                           tag="ps_s")
                            for j, kt in enumerate(kts):
                                scores_mm(h, kt, s2[:, j, :], 0)
                            e2 = es_pool.tile([P, 2, 512], BF16, name="es",
                                              tag="es")
                            nc.scalar.activation(out=e2, in_=s2, func=AF.Exp)
                            for j, kt in enumerate(kts):
                                step.append((h, kt, e2[:, j, :]))
                        flush()
                        pend.extend(step)
                    # boundary tiles, masked. Pair (kt+2, kt) so the two
                    # valid regions sit (nearly) contiguous in one [P,1024]
                    # tile and a single exp covers both with minimal N.
                    for bp in range(2):
                        ktA, ktB = nb + 2 + bp, nb + bp
                        qoffA = P * ktA - 512 * qm
                        qoffB = P * ktB - 512 * qm
                        step = []
                        for h in heads:
                            sb_ps = ps_s.tile([P, 1024], F32, name="sb",
                                              tag="ps_s")
                            scores_mm(h, ktA, sb_ps[:, qoffA:512], qoffA)
                            scores_mm(h, ktB, sb_ps[:, 512 + qoffB:], qoffB)
                            e2 = es_pool.tile([P, 1024], BF16, name="esb",
                                              tag="es")
                            # one exp covers both tiles (gap cols unread)
                            nc.scalar.activation(
                                out=e2[:, qoffA:], in_=sb_ps[:, qoffA:],
                                func=AF.Exp)
                            nc.vector.tensor_mul(
                                e2[:, qoffA:qoffA + P],
                                e2[:, qoffA:qoffA + P], tri)
                            nc.vector.tensor_mul(
                                e2[:, 512 + qoffB:512 + qoffB + P],
                                e2[:, 512 + qoffB:512 + qoffB + P], tri)
                            step.append((h, ktB, e2[:, 512 + qoffB:]))
                            step.append((h, ktA, e2[:, qoffA:512]))
                        flush()
                        pend.extend(step)
                    flush()

                    # evacuate PSUM accumulators, then normalize
                    ohu = ohu_pool.tile([P, 512], F32, name="ohu")
                    cs = cs_pool.tile([P, 2, 512], F32, name="cs")
                    for i, h in enumerate(heads):
                        hi = (h % 2) * DH
                        nc.vector.tensor_copy(
                            out=ohu[hi:hi + DH, :], in_=out_ps[h][0:DH, :])
                        nc.vector.tensor_copy(
                            out=cs[0:1, i, :], in_=out_ps[h][DH:DH + 1, :])
                    csw = cs_pool.tile([P, 8], F32, name="csw", tag="csw")
                    nc.sync.dma_start(out=csw, in_=cs[0:1, :, :])
                    nc.vector.reciprocal(out=csw, in_=csw)
                    rcs = cs_pool.tile([P, 2, 512], F32, name="rcs", tag="rcs")
                    nc.sync.dma_start(out=rcs[0:1, :, :], in_=csw)
                    rb = rb_pool.tile([P, 2, 512], F32, name="rb")
                    nc.gpsimd.partition_broadcast(rb[:, 0, :], rcs[0:1, 0, :])
                    nc.gpsimd.partition_broadcast(rb[:, 1, :], rcs[0:1, 1, :])
                    for i, h in enumerate(heads):
                        hi = (h % 2) * DH
                        nc.vector.tensor_mul(
                            oh_sb[hi:hi + DH, hp, :],
                            ohu[hi:hi + DH, :], rb[hi:hi + DH, i, :])

                    # exchange this head-pair slab with the pair core
                    if ag_batch:
                        nc.sync.dma_start(out=ohd4[hp * P:(hp + 1) * P, :],
                                          in_=oh_sb[:, hp, :])
                    else:
                        ohd = dram_pool.tile([P, 512], BF16,
                                             name=f"ohd{qm}_{hp}",
                                             tag=f"ohd{qm}_{hp}")
                        nc.sync.dma_start(out=ohd, in_=oh_sb[:, hp, :])
                        ohg = dram_pool.tile([2 * P, 512], BF16,
                                             name=f"ohg{qm}_{hp}",
                                             tag=f"ohg{qm}_{hp}")
                        nc.gpsimd.collective_compute(
                            "AllGather", mybir.AluOpType.bypass,
                            replica_groups=REPLICA_GROUPS,
                            ins=[ohd[:, :]], outs=[ohg[:, :]])
                        nc.sync.dma_start(
                            out=ohg_sb[:, :, hp, :],
                            in_=ohg.rearrange("(r p) q -> p r q", p=P))

                    # weave in dense projection work for the next chunk
                    if weave is not None:
                        for _ in range(4):
                            thunk = next(weave, None)
                            if thunk is None:
                                break
                            thunk()

                if ag_batch:
                    ohg4 = dram_pool.tile([8 * P, 512], BF16,
                                          name=f"ohg4_{qm}", tag=f"ohg4_{qm}")
                    nc.gpsimd.collective_compute(
                        "AllGather", mybir.AluOpType.bypass,
                        replica_groups=REPLICA_GROUPS,
                        ins=[ohd4[:, :]], outs=[ohg4[:, :]])
                    nc.sync.dma_start(
                        out=ohg_sb,
                        in_=ohg4.rearrange("(r s p) q -> p r s q", r=2, p=P))
                return ohg_sb

            def oc_group(qm, ohg_sb, oc):
                """This core's output cols [128*oc, 128*oc+128): full
                1024-channel contraction over both ranks' slabs."""
                ps = ps_mm.tile([P, 512], F32, name="ps_f", tag="ps_mm")
                first = True
                for rank in range(2):
                    for hp in range(4):
                        nc.tensor.matmul(
                            ps,
                            lhsT=wp2_sb[:, rank, hp, oc * P:(oc + 1) * P],
                            rhs=ohg_sb[:, rank, hp, :],
                            start=first, stop=(rank == 1 and hp == 3))
                        first = False
                po = po_pool.tile([P, 512], BF16, name="po")
                nc.vector.tensor_scalar_add(
                    out=po, in0=ps, scalar1=bias_sb[:, oc:oc + 1])
                nc.sync.dma_start(
                    out=out.ap()[P * oc:P * (oc + 1),
                                 qm * 512:(qm + 1) * 512], in_=po)

            def out_proj_groups(qm, ohg_sb):
                for oc in range(4):
                    yield (lambda qm=qm, og=ohg_sb, oc=oc:
                           oc_group(qm, og, oc))

            def out_proj(qm, ohg_sb):
                for thunk in out_proj_groups(qm, ohg_sb):
                    thunk()

            # ---- projection machinery (chunked by 512 tokens) ----
            with (
                tc.tile_pool(name="xT_pool", bufs=1) as xT_pool,
                tc.tile_pool(name="wa_pool", bufs=1) as wa_pool,
            ):
                # input loads, ordered so the V projection (wv + token
                # chunk 0) can start after ~2MB of DMA
                xT_sb = xT_pool.tile([P, ND, T], BF16)
                for qc in range(4):
                    nc.sync.dma_start(
                        out=xT_sb[:, :, qc * 512:(qc + 1) * 512],
                        in_=xT.ap()[qc])
                wv_sb = wa_pool.tile([P, ND, 512], BF16)
                nc.gpsimd.dma_start(out=wv_sb, in_=wv.ap())
                wqk_sb = wa_pool.tile([P, ND, 1024], BF16)
                nc.gpsimd.dma_start(out=wqk_sb, in_=wqk.ap())
                nc.gpsimd.dma_start(out=wp2_sb, in_=wp.ap())
                nc.gpsimd.dma_start(out=bias_sb, in_=bias.ap())
                nc.vector.memset(ones_sb, 1.0)
                nc.vector.tensor_copy(out=v1[:, :, :, DH:DH + 1], in_=ones_sb)

                def qk_group(pt, qc):
                    ps = ps_mm.tile([P, 512], F32, name="ps_qk", tag="ps_mm")
                    for dd in range(ND):
                        nc.tensor.matmul(
                            ps,
                            lhsT=wqk_sb[:, dd, pt * P:(pt + 1) * P],
                            rhs=xT_sb[:, dd, qc * 512:(qc + 1) * 512],
                            start=(dd == 0), stop=(dd == ND - 1))
                    nc.vector.tensor_copy(
                        out=qkT[:, pt, qc * 512:(qc + 1) * 512], in_=ps)

                def v_group(tt):
                    ps = ps_mm.tile([P, 512], F32, name="ps_v", tag="ps_mm")
                    for dd in range(ND):
                        nc.tensor.matmul(
                            ps,
                            lhsT=xT_sb[:, dd, tt * P:(tt + 1) * P],
                            rhs=wv_sb[:, dd, :],
                            start=(dd == 0), stop=(dd == ND - 1))
                    nc.vector.tensor_copy(
                        out=v1[:, tt, :, 0:DH],
                        in_=ps.rearrange("p (h d) -> p h d", h=HL))

                def proj_chunk_groups(qc):
                    for tt in range(4 * qc, 4 * qc + 4):
                        yield lambda tt=tt: v_group(tt)
                    for pt in range(8):
                        yield lambda pt=pt: qk_group(pt, qc)

                # prelude: project chunk 0 (V first: needs only wv+chunk 0)
                for g in proj_chunk_groups(0):
                    g()

                # q-macros 0..2, weaving in the next chunk's projections
                og2 = None
                for qm in range(3):
                    weave = proj_chunk_groups(qm + 1)
                    ohg_sb = attention(qm, weave)
                    for thunk in weave:  # drain leftovers
                        thunk()
                    if qm < 2:
                        out_proj(qm, ohg_sb)
                    else:
                        og2 = ohg_sb

            # q-macro 3: weave qm2's out-projection between its pairs
            weave3 = out_proj_groups(2, og2)
            ohg_sb = attention(3, weave3, ag_batch=True)
            for thunk in weave3:
                thunk()
            out_proj(3, ohg_sb)

    nc.finalize()
    return nc


_NC_CACHE = None


def _get_nc():
    global _NC_CACHE
    if _NC_CACHE is None:
        _NC_CACHE = build_bass()
    return _NC_CACHE


def _make_in_maps(x, w_qkv, w_proj, b_proj):
    x = np.asarray(x, np.float32)
    w_qkv = np.asarray(w_qkv, np.float32)
    w_proj = np.asarray(w_proj, np.float32)
    b_proj = np.asarray(b_proj, np.float32)
    wq, wk, wv_full = w_qkv[:, :D], w_qkv[:, D:2 * D], w_qkv[:, 2 * D:]
    scale = DH ** -0.5
    in_maps = []
    def pm(w, *shape):
        """(D, M) row-major -> partition-major SBUF layout, contiguous."""
        return np.ascontiguousarray(
            w.reshape(*shape).transpose(1, 0, 2)).astype(NP_BF16)

    for c in range(8):
        b, g = c // 2, c % 2
        cols = slice(g * 512, (g + 1) * 512)
        wqk_c = np.concatenate([wq[:, cols] * scale, wk[:, cols]], axis=1)
        # xT[qc, p, n, t'] = x[b][qc*512+t', n*128+p]
        xT_c = np.ascontiguousarray(
            x[b].T.reshape(ND, P, 4, 512).transpose(2, 1, 0, 3)
        ).astype(NP_BF16)
        # wp2[p, r, d, c'] = w_proj[r*512 + d*128 + p, cols][c']
        wp_c = np.ascontiguousarray(
            w_proj[:, cols].reshape(2, 4, P, 512).transpose(2, 0, 1, 3)
        ).astype(NP_BF16)
        in_maps.append({
            "xT": xT_c,
            "wqk": pm(wqk_c, ND, P, 1024),
            "wv": pm(wv_full[:, cols], ND, P, 512),
            "wp": wp_c,
            "bias": np.ascontiguousarray(b_proj[cols].reshape(4, P).T),
        })
    return in_maps


def _assemble(results):
    out = np.empty((B, T, D), np.float32)
    for c in range(8):
        b, g = c // 2, c % 2
        res = results[c]["out"].astype(np.float32)
        out[b, :, g * 512:(g + 1) * 512] = res.T
    return out


def kernel(x, w_qkv, w_proj, b_proj):
    nc = _get_nc()
    in_maps = _make_in_maps(x, w_qkv, w_proj, b_proj)
    res = run_bass_kernel_spmd(nc, in_maps, core_ids=list(range(8)))
    return _assemble(res.results)


def kernel_traced(x, w_qkv, w_proj, b_proj, **kw):
    """Like kernel() but returns (output, BassKernelResults) with trace."""
    nc = _get_nc()
    in_maps = _make_in_maps(x, w_qkv, w_proj, b_proj)
    res = run_bass_kernel_spmd(nc, in_maps, core_ids=list(range(8)),
                               trace=True, **kw)
    return _assemble(res.results), res


# revision 28
# speedup vs baseline: 1.4597x; 1.0461x over previous
"""Multi-head causal attention (B=4, T=2048, D=1024, H=16) on 8 Trainium2
NeuronCores.

Sharding: core c -> (batch = c//2, head-group g = c%2, 8 heads each).
Each core: QKV projection for its batch/head-group, causal attention,
then a per-head-pair 2-way AllGather of the attention output with its
pair core (same batch, other head-group), followed by a fully LOCAL
output projection over this core's 512 output channels (full 1024-
channel contraction) written straight to out. No ReduceScatter, no
partial staging: the only collective after the last attention matmul
is one 128KB AllGather. Host reassembles with a transpose+concat only.

On-chip orientation is "transposed" throughout (channels on partitions,
tokens on the free dim):
  xT   (D, T)    bf16, pre-transposed on HOST (keeps PE warm from t=0)
  qkT  (1024, T) = wqk.T @ xT  (q rows pre-scaled by 1/sqrt(Dh) on host)
  v    (T, 512)  natural, with an extra all-ones column per head slot
  sT   (k, q)    = K_tile @ qT  -> exp on ScalarE -> es (bf16)
  outT (65, q)   = [v|1].T @ es  (fp32 PSUM; row 64 = softmax denom)
  oh   (512, q)  normalized attention out, AllGathered per 128-row
                 head-pair slab as soon as each head-pair completes
  final (512own, q) = wp_all.T @ ohAG + bias  -> DMA to out

The QKV projection is produced in 512-token chunks and chunk qc+1 is
woven between the attention head-pairs of q-macro qc, so the PE stream
stays dense (exp latency on ScalarE is hidden by projection matmuls)
and the PE clock stays un-throttled. AV matmuls run one pipeline step
behind scores/exp. Causal mask: strictly-upper triangular 128x128
multiply on boundary tiles only; future k-tiles are never computed.
Matmul operands are bf16 (1 PE cycle/row, fp32 accumulate); softmax
statistics stay fp32; exchanged slabs are bf16.
"""

import numpy as np
import ml_dtypes

import concourse.bass as bass
from concourse import bacc
import concourse.mybir as mybir
import concourse.tile as tile
from concourse.bass_utils import run_bass_kernel_spmd
from concourse.masks import make_upper_triangular

B, T, D = 4, 2048, 1024
H_TOT, DH = 16, 64
HL = 8          # heads per core
P = 128
ND = D // P     # 8 d-tiles
NT = T // P     # 16 token tiles
NQ = T // 512   # 4 q-macros
F32 = mybir.dt.float32
BF16 = mybir.dt.bfloat16
AF = mybir.ActivationFunctionType
NP_BF16 = ml_dtypes.bfloat16

REPLICA_GROUPS = [[0, 1], [2, 3], [4, 5], [6, 7]]


def build_bass():
    nc = bacc.Bacc(None, target_bir_lowering=False, num_devices=8)

    # all inputs pre-permuted on host into SBUF layout (partition-major)
    # so every input DMA is a contiguous read
    xT = nc.dram_tensor("xT", [4, P, ND, 512], BF16, kind="ExternalInput")
    wqk = nc.dram_tensor("wqk", [P, ND, 1024], BF16, kind="ExternalInput")
    wv = nc.dram_tensor("wv", [P, ND, 512], BF16, kind="ExternalInput")
    # full 1024 contraction rows x this core's 512 output cols
    wp = nc.dram_tensor("wp", [P, 2, 4, 512], BF16, kind="ExternalInput")
    bias = nc.dram_tensor("bias", [P, 4], F32, kind="ExternalInput")
    out = nc.dram_tensor("out", [512, T], BF16, kind="ExternalOutput")

    with tile.TileContext(nc, num_cores=8) as tc:
        with (
            tc.tile_pool(name="const", bufs=1) as const_pool,
            tc.tile_pool(name="dram", bufs=1, space="DRAM") as dram_pool,
            tc.tile_pool(name="persist", bufs=1) as persist,
            tc.tile_pool(name="wp_pool", bufs=1) as wp_pool,
            tc.tile_pool(name="es_pool", bufs=10) as es_pool,
            tc.tile_pool(name="oh_pool", bufs=2) as oh_pool,
            tc.tile_pool(name="og_pool", bufs=2) as og_pool,
            tc.tile_pool(name="ohu_pool", bufs=3) as ohu_pool,
            tc.tile_pool(name="cs_pool", bufs=2) as cs_pool,
            tc.tile_pool(name="rb_pool", bufs=3) as rb_pool,
            tc.tile_pool(name="po_pool", bufs=3) as po_pool,
            tc.tile_pool(name="ps_s", bufs=2, space="PSUM") as ps_s,
            tc.tile_pool(name="ps_av", bufs=2, space="PSUM") as ps_av,
            tc.tile_pool(name="ps_mm", bufs=2, space="PSUM") as ps_mm,
        ):
            tri = const_pool.tile([P, P], BF16)
            bias_sb = const_pool.tile([P, 4], F32)

            qkT = persist.tile([P, ND, T], BF16, name="qkT")
            v1 = persist.tile([P, NT, HL, DH + 1], BF16, name="v1")
            ones_sb = const_pool.tile([P, NT, HL, 1], F32)
            # wp2[p, rank, hp, c] = w_proj[rank*512 + hp*128 + p, mycols[c]]
            wp2_sb = wp_pool.tile([P, 2, 4, 512], BF16)

            def attention(qm, weave):
                """Attention for q-macro qm; pulls from `weave` (an iterator
                of thunks emitting projection matmul groups) between pairs.
                Head-pair slabs are staged to DRAM as they finish and
                exchanged with the pair core in ONE AllGather at the end
                (the SBUF copy-back is deferred to out_proj so no engine
                queue ever head-of-line blocks on collective completion)."""
                nkt = 4 * qm + 4
                nb = nkt - 4
                oh_sb = oh_pool.tile([P, 4, 512], BF16, name="oh_sb")
                ohd4 = dram_pool.tile([4 * P, 512], BF16,
                                      name=f"ohd4_{qm}", tag=f"ohd4_{qm}")

                for hp in range(4):
                    heads = (2 * hp, 2 * hp + 1)
                    out_ps = {}
                    av_issued = {}
                    for h in heads:
                        out_ps[h] = ps_av.tile([DH + 1, 512], F32,
                                               name=f"out_ps{h % 2}",
                                               tag="ps_av")
                        av_issued[h] = 0

                    def av_mm(h, kt, src):
                        nc.tensor.matmul(
                            out_ps[h][:, max(0, P * kt - 512 * qm):],
                            lhsT=v1[:, kt, h, :],
                            rhs=src,
                            start=(av_issued[h] == 0),
                            stop=(av_issued[h] == nkt - 1),
                            skip_group_check=True)
                        av_issued[h] += 1

                    def scores_mm(h, kt, dst, qoff):
                        hi = (h % 2) * DH
                        nc.tensor.matmul(
                            dst,
                            lhsT=qkT[hi:hi + DH, 4 + h // 2,
                                     kt * P:(kt + 1) * P],
                            rhs=qkT[hi:hi + DH, h // 2,
                                    qm * 512 + qoff:(qm + 1) * 512],
                            start=True, stop=True)

                    # software pipeline: AVs one step behind scores/exp
                    pend = []

                    def flush():
                        for h_, kt_, src_ in pend:
                            av_mm(h_, kt_, src_)
                        pend.clear()

                    for kt2 in range(nb // 2):   # non-boundary, 2 per exp
                        kts = (2 * kt2, 2 * kt2 + 1)
                        step = []
                        for h in heads:
                            s2 = ps_s.tile([P, 2, 512], F32, name="s2",
                                           tag="ps_s")
                            for j, kt in enumerate(kts):
                                scores_mm(h, kt, s2[:, j, :], 0)
                            e2 = es_pool.tile([P, 2, 512], BF16, name="es",
                                              tag="es")
                            nc.scalar.activation(out=e2, in_=s2, func=AF.Exp)
                            for j, kt in enumerate(kts):
                                step.append((h, kt, e2[:, j, :]))
                        flush()
                        pend.extend(step)
                    # boundary tiles, masked. Pair (kt+2, kt) so the two
                    # valid regions sit (nearly) contiguous in one [P,1024]
                    # tile and a single exp covers both with minimal N.
                    for bp in range(2):
                        ktA, ktB = nb + 2 + bp, nb + bp
                        qoffA = P * ktA - 512 * qm
                        qoffB = P * ktB - 512 * qm
                        step = []
                        for h in heads:
                            sb_ps = ps_s.tile([P, 1024], F32, name="sb",
                                              tag="ps_s")
                            scores_mm(h, ktA, sb_ps[:, qoffA:512], qoffA)
                            scores_mm(h, ktB, sb_ps[:, 512 + qoffB:], qoffB)
                            e2 = es_pool.tile([P, 1024], BF16, name="esb",
                                              tag="es")
                            # one exp covers both tiles (gap cols unread)
                            nc.scalar.activation(
                                out=e2[:, qoffA:], in_=sb_ps[:, qoffA:],
                                func=AF.Exp)
                            nc.vector.tensor_mul(
                                e2[:, qoffA:qoffA + P],
                                e2[:, qoffA:qoffA + P], tri)
                            nc.vector.tensor_mul(
                                e2[:, 512 + qoffB:512 + qoffB + P],
                                e2[:, 512 + qoffB:512 + qoffB + P], tri)
                            step.append((h, ktB, e2[:, 512 + qoffB:]))
                            step.append((h, ktA, e2[:, qoffA:512]))
                        flush()
                        pend.extend(step)
                    flush()

                    # evacuate PSUM accumulators, then normalize
                    ohu = ohu_pool.tile([P, 512], F32, name="ohu")
                    cs = cs_pool.tile([P, 2, 512], F32, name="cs")
                    for i, h in enumerate(heads):
                        hi = (h % 2) * DH
                        nc.vector.tensor_copy(
                            out=ohu[hi:hi + DH, :], in_=out_ps[h][0:DH, :])
                        nc.vector.tensor_copy(
                            out=cs[0:1, i, :], in_=out_ps[h][DH:DH + 1, :])
                    csw = cs_pool.tile([P, 8], F32, name="csw", tag="csw")
                    nc.gpsimd.dma_start(out=csw, in_=cs[0:1, :, :])
                    nc.vector.reciprocal(out=csw, in_=csw)
                    rcs = cs_pool.tile([P, 2, 512], F32, name="rcs", tag="rcs")
                    nc.gpsimd.dma_start(out=rcs[0:1, :, :], in_=csw)
                    rb = rb_pool.tile([P, 2, 512], F32, name="rb")
                    nc.gpsimd.partition_broadcast(rb[:, 0, :], rcs[0:1, 0, :])
                    nc.gpsimd.partition_broadcast(rb[:, 1, :], rcs[0:1, 1, :])
                    for i, h in enumerate(heads):
                        hi = (h % 2) * DH
                        nc.vector.tensor_mul(
                            oh_sb[hi:hi + DH, hp, :],
                            ohu[hi:hi + DH, :], rb[hi:hi + DH, i, :])

                    # stage this head-pair slab for the end-of-qm exchange
                    nc.sync.dma_start(out=ohd4[hp * P:(hp + 1) * P, :],
                                      in_=oh_sb[:, hp, :])

                    # weave in dense projection work for the next chunk
                    if weave is not None:
                        for _ in range(4):
                            thunk = next(weave, None)
                            if thunk is None:
                                break
                            thunk()

                ohg4 = dram_pool.tile([8 * P, 512], BF16,
                                      name=f"ohg4_{qm}", tag=f"ohg4_{qm}")
                nc.gpsimd.collective_compute(
                    "AllGather", mybir.AluOpType.bypass,
                    replica_groups=REPLICA_GROUPS,
                    ins=[ohd4[:, :]], outs=[ohg4[:, :]])
                return ohg4

            def oc_group(qm, ohg_sb, oc):
                """This core's output cols [128*oc, 128*oc+128): full
                1024-channel contraction over both ranks' slabs."""
                ps = ps_mm.tile([P, 512], F32, name="ps_f", tag="ps_mm")
                first = True
                for rank in range(2):
                    for hp in range(4):
                        nc.tensor.matmul(
                            ps,
                            lhsT=wp2_sb[:, rank, hp, oc * P:(oc + 1) * P],
                            rhs=ohg_sb[:, rank, hp, :],
                            start=first, stop=(rank == 1 and hp == 3))
                        first = False
                po = po_pool.tile([P, 512], BF16, name="po")
                nc.vector.tensor_scalar_add(
                    out=po, in0=ps, scalar1=bias_sb[:, oc:oc + 1])
                nc.sync.dma_start(
                    out=out.ap()[P * oc:P * (oc + 1),
                                 qm * 512:(qm + 1) * 512], in_=po)

            def out_proj_groups(qm, ohg4):
                """First thunk copies the AllGathered slabs back to SBUF;
                the rest project 128 output cols each."""
                ohg_sb = og_pool.tile([P, 2, 4, 512], BF16, name="ohg_sb")

                def back():
                    nc.sync.dma_start(
                        out=ohg_sb,
                        in_=ohg4.rearrange("(r s p) q -> p r s q", r=2, p=P))
                yield back
                for oc in range(4):
                    yield (lambda qm=qm, og=ohg_sb, oc=oc:
                           oc_group(qm, og, oc))

            def out_proj(qm, ohg4):
                for thunk in out_proj_groups(qm, ohg4):
                    thunk()

            # ---- projection machinery (chunked by 512 tokens) ----
            with (
                tc.tile_pool(name="xT_pool", bufs=1) as xT_pool,
                tc.tile_pool(name="wa_pool", bufs=1) as wa_pool,
            ):
                # input loads, ordered so the V projection (wv + token
                # chunk 0) can start after ~2MB of DMA
                xT_sb = xT_pool.tile([P, ND, T], BF16)
                for qc in range(4):
                    nc.sync.dma_start(
                        out=xT_sb[:, :, qc * 512:(qc + 1) * 512],
                        in_=xT.ap()[qc])
                wv_sb = wa_pool.tile([P, ND, 512], BF16)
                nc.gpsimd.dma_start(out=wv_sb, in_=wv.ap())
                wqk_sb = wa_pool.tile([P, ND, 1024], BF16)
                nc.gpsimd.dma_start(out=wqk_sb, in_=wqk.ap())
                nc.gpsimd.dma_start(out=wp2_sb, in_=wp.ap())
                nc.gpsimd.dma_start(out=bias_sb, in_=bias.ap())
                nc.vector.memset(ones_sb, 1.0)
                nc.vector.tensor_copy(out=v1[:, :, :, DH:DH + 1], in_=ones_sb)
                make_upper_triangular(nc, tri, val=1.0, diag=True)

                def qk_group(pt, qc):
                    ps = ps_mm.tile([P, 512], F32, name="ps_qk", tag="ps_mm")
                    for dd in range(ND):
                        nc.tensor.matmul(
                            ps,
                            lhsT=wqk_sb[:, dd, pt * P:(pt + 1) * P],
                            rhs=xT_sb[:, dd, qc * 512:(qc + 1) * 512],
                            start=(dd == 0), stop=(dd == ND - 1))
                    nc.vector.tensor_copy(
                        out=qkT[:, pt, qc * 512:(qc + 1) * 512], in_=ps)

                def v_group(tt):
                    ps = ps_mm.tile([P, 512], F32, name="ps_v", tag="ps_mm")
                    for dd in range(ND):
                        nc.tensor.matmul(
                            ps,
                            lhsT=xT_sb[:, dd, tt * P:(tt + 1) * P],
                            rhs=wv_sb[:, dd, :],
                            start=(dd == 0), stop=(dd == ND - 1))
                    nc.vector.tensor_copy(
                        out=v1[:, tt, :, 0:DH],
                        in_=ps.rearrange("p (h d) -> p h d", h=HL))

                def proj_chunk_groups(qc):
                    for tt in range(4 * qc, 4 * qc + 4):
                        yield lambda tt=tt: v_group(tt)
                    for pt in range(8):
                        yield lambda pt=pt: qk_group(pt, qc)

                # prelude: project chunk 0 (V first: needs only wv+chunk 0)
                for g in proj_chunk_groups(0):
                    g()

                # q-macros 0..2: weave the next chunk's projections and
                # the PREVIOUS q-macro's out-projection into each one
                from itertools import chain as _chain
                og_prev = None
                for qm in range(3):
                    weave = proj_chunk_groups(qm + 1)
                    if og_prev is not None:
                        weave = _chain(weave,
                                       out_proj_groups(qm - 1, og_prev))
                    og_prev = attention(qm, weave)
                    for thunk in weave:  # drain leftovers
                        thunk()

            # q-macro 3: weave qm2's out-projection between its pairs
            weave3 = out_proj_groups(2, og_prev)
            ohg4_3 = attention(3, weave3)
            for thunk in weave3:
                thunk()
            # keep the PE clock warm while the final AllGather drains
            # (idle > 3.4us re-throttles it to 1.2GHz right before the
            # last 32 projection matmuls); the copy gives the PSUM group
            # a reader so the pool slot releases
            warm_ps = ps_mm.tile([P, 512], F32, name="warm", tag="ps_mm")
            for w in range(36):
                nc.tensor.matmul(
                    warm_ps,
                    lhsT=wp2_sb[:, 0, 0, 0:P],
                    rhs=wp2_sb[:, 0, w % 4, 0:512],
                    start=(w == 0), stop=(w == 35))
            warm_sink = po_pool.tile([P, 8], F32, name="warm_sink")
            nc.vector.tensor_copy(out=warm_sink, in_=warm_ps[:, 0:8])
            out_proj(3, ohg4_3)

    nc.finalize()
    return nc


_NC_CACHE = None


def _get_nc():
    global _NC_CACHE
    if _NC_CACHE is None:
        _NC_CACHE = build_bass()
    return _NC_CACHE


def _make_in_maps(x, w_qkv, w_proj, b_proj):
    x = np.asarray(x, np.float32)
    w_qkv = np.asarray(w_qkv, np.float32)
    w_proj = np.asarray(w_proj, np.float32)
    b_proj = np.asarray(b_proj, np.float32)
    wq, wk, wv_full = w_qkv[:, :D], w_qkv[:, D:2 * D], w_qkv[:, 2 * D:]
    scale = DH ** -0.5
    in_maps = []
    def pm(w, *shape):
        """(D, M) row-major -> partition-major SBUF layout, contiguous."""
        return np.ascontiguousarray(
            w.reshape(*shape).transpose(1, 0, 2)).astype(NP_BF16)

    for c in range(8):
        b, g = c // 2, c % 2
        cols = slice(g * 512, (g + 1) * 512)
        wqk_c = np.concatenate([wq[:, cols] * scale, wk[:, cols]], axis=1)
        # xT[qc, p, n, t'] = x[b][qc*512+t', n*128+p]
        xT_c = np.ascontiguousarray(
            x[b].T.reshape(ND, P, 4, 512).transpose(2, 1, 0, 3)
        ).astype(NP_BF16)
        # wp2[p, r, d, c'] = w_proj[r*512 + d*128 + p, cols][c']
        wp_c = np.ascontiguousarray(
            w_proj[:, cols].reshape(2, 4, P, 512).transpose(2, 0, 1, 3)
        ).astype(NP_BF16)
        in_maps.append({
            "xT": xT_c,
            "wqk": pm(wqk_c, ND, P, 1024),
            "wv": pm(wv_full[:, cols], ND, P, 512),
            "wp": wp_c,
            "bias": np.ascontiguousarray(b_proj[cols].reshape(4, P).T),
        })
    return in_maps


def _assemble(results):
    out = np.empty((B, T, D), np.float32)
    for c in range(8):
        b, g = c // 2, c % 2
        res = results[c]["out"].astype(np.float32)
        out[b, :, g * 512:(g + 1) * 512] = res.T
    return out


def kernel(x, w_qkv, w_proj, b_proj):
    nc = _get_nc()
    in_maps = _make_in_maps(x, w_qkv, w_proj, b_proj)
    res = run_bass_kernel_spmd(nc, in_maps, core_ids=list(range(8)))
    return _assemble(res.results)


def kernel_traced(x, w_qkv, w_proj, b_proj, **kw):
    """Like kernel() but returns (output, BassKernelResults) with trace."""
    nc = _get_nc()
    in_maps = _make_in_maps(x, w_qkv, w_proj, b_proj)
    res = run_bass_kernel_spmd(nc, in_maps, core_ids=list(range(8)),
                               trace=True, **kw)
    return _assemble(res.results), res
